# revision 19
# baseline (speedup 1.0000x reference)
"""GCN layer (gather + segment_sum + scale) on 8 Trainium2 NeuronCores.

Strategy (1D destination-node parallel):
  - Host (integer/index work only): shard edges by dst block of 12500 nodes
    (core i owns dst nodes [12500*i, 12500*(i+1))). Per core, sort owned
    nodes by in-degree, pack them into 98 groups of 128 nodes. Group g gets
    k_g "slot tiles": slot (p, c) holds the j-th in-edge of node rank
    g*128+p, padded with a pointer to an all-zero table row when j >= deg.
    Groups with equal k are batched so device-side adds are wide.
  - Device (all FP math): table rows are [node_f || out_d] (33 f32).
    Chunked indirect-DMA gather (SWDGE) of 128-column slot blocks ->
    DVE scale-by-out_d (pack to 32 wide) -> DVE wide accumulate into an
    SBUF accumulator [128, 98, 32] -> multiply by in_dg -> DMA out.
  - Host: inverse-permute rows back to original node order.
"""

import sys

import numpy as np

for _p in ("/opt/trn_rl_repo", "/root/.axon_site/_ro/trn_rl_repo"):
    if _p not in sys.path:
        sys.path.append(_p)

P = 128
D = 32
TW = D + 1  # table row width: 32 features + out_d
CHUNK_COLS = 128
N_CORES = 8

_cache = {}


# ---------------------------------------------------------------- host prep


def _segments(k_g, chunk_cols):
    """List of (chunk, cs_local, w, gs, j): add (or copy when j==0)
    pk[chunk][:, cs:cs+w, :] into acc[:, gs:gs+w, :]."""
    G = len(k_g)
    segs = []
    c = 0
    g0 = 0
    colbase = {}
    while g0 < G:
        g1 = g0
        while g1 < G and k_g[g1] == k_g[g0]:
            g1 += 1
        k = int(k_g[g0])
        b = g1 - g0
        for j in range(k):
            colbase[(g0, j)] = c
            s = c
            while s < c + b:
                e = min(c + b, (s // chunk_cols + 1) * chunk_cols)
                # last=True marks the final accumulation into this group range:
                # finalize (in_dg multiply + output DMA) can fire right after.
                segs.append(
                    (s // chunk_cols, s % chunk_cols, e - s, g0 + (s - c), j, j == k - 1)
                )
                s = e
            c += b
        g0 = g1
    return segs, colbase, c


def _preprocess(node_f, out_d, in_dg, src, dst):
    n = node_f.shape[0]
    npc = n // N_CORES  # nodes per core
    G = (npc + P - 1) // P
    node_slots = G * P

    table = np.zeros((n + 1, TW), dtype=np.float32)
    table[:n, :D] = node_f
    table[:n, D] = out_d[:, 0]

    core_of = dst // npc
    per_core = []
    deg_all = []
    for i in range(N_CORES):
        m = core_of == i
        e_src = src[m].astype(np.int64)
        e_dstl = (dst[m] - i * npc).astype(np.int64)
        # group edges by local dst
        perm = np.argsort(e_dstl, kind="stable")
        e_src = e_src[perm]
        e_dstl = e_dstl[perm]
        deg = np.bincount(e_dstl, minlength=npc)
        deg_ext = np.zeros(node_slots, dtype=np.int64)
        deg_ext[:npc] = deg
        order = np.argsort(deg_ext, kind="stable")  # rank -> node slot
        per_core.append((e_src, e_dstl, deg, deg_ext, order))
        deg_all.append(deg_ext[order])  # sorted degrees

    # global slot count per group = max over cores of group-max degree
    k_g = np.zeros(G, dtype=np.int64)
    for i in range(N_CORES):
        sd = deg_all[i]
        k_g = np.maximum(k_g, sd.reshape(G, P).max(axis=1))
    # >=1 so every acc region gets its j==0 copy (first-touch init)
    k_g = np.maximum(k_g, 1)

    segs, colbase, C = _segments(k_g, CHUNK_COLS)
    C_pad = ((C + CHUNK_COLS - 1) // CHUNK_COLS) * CHUNK_COLS

    # colOf lookup table for vectorized idx fill
    kmax = int(k_g.max())
    colOf = np.full((G, kmax), C_pad - 1, dtype=np.int64)  # default -> harmless
    g = 0
    while g < G:
        g1 = g
        while g1 < G and k_g[g1] == k_g[g]:
            g1 += 1
        for j in range(int(k_g[g])):
            colOf[g:g1, j] = colbase[(g, j)] + np.arange(g1 - g)
        g = g1

    idx_arrs = np.full((N_CORES, P, C_pad), n, dtype=np.int32)  # n = zero row
    indg_arrs = np.zeros((N_CORES, P, G, 1), dtype=np.float32)
    orders = []
    for i in range(N_CORES):
        e_src, e_dstl, deg, deg_ext, order = per_core[i]
        rank_of = np.empty(node_slots, dtype=np.int64)
        rank_of[order] = np.arange(node_slots)
        off = np.zeros(npc + 1, dtype=np.int64)
        np.cumsum(deg, out=off[1:])
        j_e = np.arange(len(e_src)) - off[e_dstl]
        r_e = rank_of[e_dstl]
        col_e = colOf[r_e // P, j_e]
        idx_arrs[i, r_e % P, col_e] = e_src
        rr = np.arange(node_slots)
        real = order < npc
        indg_arrs[i, rr[real] % P, rr[real] // P, 0] = in_dg[i * npc + order[real], 0]
        orders.append(order)

    return dict(
        table=table,
        idx=idx_arrs,
        indg=indg_arrs,
        orders=orders,
        segs=segs,
        C_pad=C_pad,
        G=G,
        npc=npc,
        n=n,
        slots_real=int(sum(len(pc[0]) for pc in per_core)),
        slots_total=int(N_CORES * P * C_pad),
    )


# ---------------------------------------------------------------- device


def _build_nc(n_table_rows, C_pad, G, segs):
    import concourse.bass as bass
    import concourse.tile as tile
    from concourse import bacc, mybir

    nc = bacc.Bacc("TRN2", target_bir_lowering=False, debug=False)
    table_d = nc.dram_tensor(
        "table", [n_table_rows, TW], mybir.dt.float32, kind="ExternalInput"
    ).ap()
    idx_d = nc.dram_tensor("idx", [P, C_pad], mybir.dt.int32, kind="ExternalInput").ap()
    indg_d = nc.dram_tensor(
        "indg", [P, G, 1], mybir.dt.float32, kind="ExternalInput"
    ).ap()
    out_dram = nc.dram_tensor(
        "out", [P, G, D], mybir.dt.float32, kind="ExternalOutput"
    ).ap()

    n_chunks = C_pad // CHUNK_COLS
    segs_by_chunk = {}
    for (ch, cs, w, gs, j, last) in segs:
        segs_by_chunk.setdefault(ch, []).append((cs, w, gs, j, last))

    with tile.TileContext(nc) as tc:
        with (
            tc.tile_pool(name="persist", bufs=1) as persist,
            tc.tile_pool(name="idxp", bufs=4) as idx_pool,
            tc.tile_pool(name="msgs", bufs=4) as msgs_pool,
            tc.tile_pool(name="packed", bufs=3) as packed_pool,
        ):
            indg_t = persist.tile([P, G, 1], mybir.dt.float32)
            acc = persist.tile([P, G, D], mybir.dt.float32)
            nc.sync.dma_start(out=indg_t[:], in_=indg_d[:])
            for ch in range(n_chunks):
                # just-in-time index load: chunk 0's gathers start after a
                # 64KB transfer instead of the whole index array
                idx_t = idx_pool.tile([P, CHUNK_COLS], mybir.dt.int32, tag="ix")
                nc.sync.dma_start(
                    out=idx_t[:],
                    in_=idx_d[:, ch * CHUNK_COLS : (ch + 1) * CHUNK_COLS],
                )
                m = msgs_pool.tile([P, CHUNK_COLS, TW], mybir.dt.float32, tag="m")
                for c in range(CHUNK_COLS):
                    # HW indirect DMA semantics: one descriptor per partition,
                    # row = table[idx[p]] -> dest partition line (2D AP only).
                    nc.gpsimd.indirect_dma_start(
                        out=m[:, c, :],
                        out_offset=None,
                        in_=table_d[:],
                        in_offset=bass.IndirectOffsetOnAxis(
                            ap=idx_t[:, c : c + 1], axis=0
                        ),
                    )
                pk = packed_pool.tile([P, CHUNK_COLS, D], mybir.dt.float32, tag="pk")
                nc.vector.tensor_tensor(
                    out=pk[:],
                    in0=m[:, :, 0:D],
                    in1=m[:, :, D : D + 1].to_broadcast([P, CHUNK_COLS, D]),
                    op=mybir.AluOpType.mult,
                )
                for (cs, w, gs, j, last) in segs_by_chunk.get(ch, []):
                    if j == 0:
                        # first tile of the group: init the acc region
                        nc.vector.tensor_copy(
                            out=acc[:, gs : gs + w, :], in_=pk[:, cs : cs + w, :]
                        )
                    else:
                        nc.vector.tensor_tensor(
                            out=acc[:, gs : gs + w, :],
                            in0=pk[:, cs : cs + w, :],
                            in1=acc[:, gs : gs + w, :],
                            op=mybir.AluOpType.add,
                        )
                    if last:
                        # final accumulation for groups [gs, gs+w): scale by
                        # in_dg and ship out now, overlapping later chunks
                        nc.vector.tensor_tensor(
                            out=acc[:, gs : gs + w, :],
                            in0=acc[:, gs : gs + w, :],
                            in1=indg_t[:, gs : gs + w, :].to_broadcast([P, w, D]),
                            op=mybir.AluOpType.mult,
                        )
                        nc.sync.dma_start(
                            out=out_dram[:, gs : gs + w, :],
                            in_=acc[:, gs : gs + w, :],
                        )
    nc.compile()
    return nc


# ---------------------------------------------------------------- entry


last_run_info = {}


def kernel(node_f, out_d, in_dg, src, dst, *, _trace=False):
    node_f = np.asarray(node_f, dtype=np.float32)
    out_d = np.asarray(out_d, dtype=np.float32)
    in_dg = np.asarray(in_dg, dtype=np.float32)
    src = np.asarray(src)
    dst = np.asarray(dst)

    pp = _preprocess(node_f, out_d, in_dg, src, dst)

    key = (pp["n"], pp["C_pad"], pp["G"], len(pp["segs"]))
    if key not in _cache:
        _cache.clear()
        _cache[key] = _build_nc(pp["n"] + 1, pp["C_pad"], pp["G"], pp["segs"])
    nc = _cache[key]

    from concourse.bass_utils import run_bass_kernel_spmd

    in_maps = [
        {"table": pp["table"], "idx": pp["idx"][i], "indg": pp["indg"][i]}
        for i in range(N_CORES)
    ]
    # Sacrificial device touch: after an earlier crashed session the first
    # device interaction can report NRT_EXEC_UNIT_UNRECOVERABLE once and
    # then recover; absorb that here instead of failing the real run.
    try:
        import jax
        import jax.numpy as jnp

        jnp.zeros((2,)).block_until_ready()
    except Exception:
        pass

    trace_kwargs = (
        dict(trace=True, trace_cores=list(range(N_CORES))) if _trace else {}
    )
    res = None
    last_exc = None
    for attempt in range(3):
        try:
            res = run_bass_kernel_spmd(
                nc, in_maps, core_ids=list(range(N_CORES)), **trace_kwargs
            )
            break
        except ModuleNotFoundError:
            # NTFF profiling hook unavailable in this environment
            trace_kwargs = {}
        except Exception as e:  # noqa: BLE001
            last_exc = e
            import time as _time

            _time.sleep(2.0)
    if res is None:
        res = run_bass_kernel_spmd(nc, in_maps, core_ids=list(range(N_CORES)))
    last_run_info["exec_time_ns"] = res.exec_time_ns
    last_run_info["mean_exec_time_ns"] = res.mean_exec_time_ns
    last_run_info["trace"] = res.instructions_and_trace
    last_run_info["pp_stats"] = {
        k: pp[k] for k in ("C_pad", "G", "slots_real", "slots_total")
    }

    n, npc, G = pp["n"], pp["npc"], pp["G"]
    out = np.empty((n, D), dtype=np.float32)
    rr = np.arange(G * P)
    for i in range(N_CORES):
        o = res.results[i]["out"]  # [P, G, D]
        order = pp["orders"][i]
        real = order < npc
        out[i * npc + order[real]] = o[rr[real] % P, rr[real] // P]
    return out


# revision 23
# speedup vs baseline: 1.0474x; 1.0474x over previous
"""GCN layer (gather + segment_sum + scale) on 8 Trainium2 NeuronCores.

Strategy (1D destination-node parallel):
  - Host (integer/index work only): shard edges by dst block of 12500 nodes
    (core i owns dst nodes [12500*i, 12500*(i+1))). Per core, sort owned
    nodes by in-degree, pack them into 98 groups of 128 nodes. Group g gets
    k_g "slot tiles": slot (p, c) holds the j-th in-edge of node rank
    g*128+p, padded with a pointer to an all-zero table row when j >= deg.
    Groups with equal k are batched so device-side adds are wide.
  - Device (all FP math): table rows are [node_f || out_d] (33 f32).
    Chunked indirect-DMA gather (SWDGE) of 128-column slot blocks ->
    DVE scale-by-out_d (pack to 32 wide) -> DVE wide accumulate into an
    SBUF accumulator [128, 98, 32] -> multiply by in_dg -> DMA out.
  - Host: inverse-permute rows back to original node order.
"""

import sys

import numpy as np

for _p in ("/opt/trn_rl_repo", "/root/.axon_site/_ro/trn_rl_repo"):
    if _p not in sys.path:
        sys.path.append(_p)

P = 128
D = 32
TW = D + 1  # table row width: 32 features + out_d
CHUNK_COLS = 128
N_CORES = 8

_cache = {}


# ---------------------------------------------------------------- host prep


def _segments(k_g, chunk_cols):
    """List of (chunk, cs_local, w, gs, j): add (or copy when j==0)
    pk[chunk][:, cs:cs+w, :] into acc[:, gs:gs+w, :]."""
    G = len(k_g)
    segs = []
    c = 0
    g0 = 0
    colbase = {}
    while g0 < G:
        g1 = g0
        while g1 < G and k_g[g1] == k_g[g0]:
            g1 += 1
        k = int(k_g[g0])
        b = g1 - g0
        for j in range(k):
            colbase[(g0, j)] = c
            s = c
            while s < c + b:
                e = min(c + b, (s // chunk_cols + 1) * chunk_cols)
                # last=True marks the final accumulation into this group range:
                # finalize (in_dg multiply + output DMA) can fire right after.
                segs.append(
                    (s // chunk_cols, s % chunk_cols, e - s, g0 + (s - c), j, j == k - 1)
                )
                s = e
            c += b
        g0 = g1
    return segs, colbase, c


def _preprocess(node_f, out_d, in_dg, src, dst):
    n = node_f.shape[0]
    npc = n // N_CORES  # nodes per core
    G = (npc + P - 1) // P
    node_slots = G * P

    table = np.zeros((n + 1, TW), dtype=np.float32)
    table[:n, :D] = node_f
    table[:n, D] = out_d[:, 0]

    core_of = dst // npc
    per_core = []
    deg_all = []
    for i in range(N_CORES):
        m = core_of == i
        e_src = src[m].astype(np.int64)
        e_dstl = (dst[m] - i * npc).astype(np.int64)
        # group edges by local dst
        perm = np.argsort(e_dstl, kind="stable")
        e_src = e_src[perm]
        e_dstl = e_dstl[perm]
        deg = np.bincount(e_dstl, minlength=npc)
        deg_ext = np.zeros(node_slots, dtype=np.int64)
        deg_ext[:npc] = deg
        order = np.argsort(deg_ext, kind="stable")  # rank -> node slot
        per_core.append((e_src, e_dstl, deg, deg_ext, order))
        deg_all.append(deg_ext[order])  # sorted degrees

    # global slot count per group = max over cores of group-max degree
    k_g = np.zeros(G, dtype=np.int64)
    for i in range(N_CORES):
        sd = deg_all[i]
        k_g = np.maximum(k_g, sd.reshape(G, P).max(axis=1))
    # >=1 so every acc region gets its j==0 copy (first-touch init)
    k_g = np.maximum(k_g, 1)

    segs, colbase, C = _segments(k_g, CHUNK_COLS)
    C_pad = C  # no chunk rounding: the last chunk is emitted partial-width

    # colOf lookup table for vectorized idx fill
    kmax = int(k_g.max())
    colOf = np.full((G, kmax), C_pad - 1, dtype=np.int64)  # default -> harmless
    g = 0
    while g < G:
        g1 = g
        while g1 < G and k_g[g1] == k_g[g]:
            g1 += 1
        for j in range(int(k_g[g])):
            colOf[g:g1, j] = colbase[(g, j)] + np.arange(g1 - g)
        g = g1

    idx_arrs = np.full((N_CORES, P, C_pad), n, dtype=np.int32)  # n = zero row
    indg_arrs = np.zeros((N_CORES, P, G, 1), dtype=np.float32)
    orders = []
    for i in range(N_CORES):
        e_src, e_dstl, deg, deg_ext, order = per_core[i]
        rank_of = np.empty(node_slots, dtype=np.int64)
        rank_of[order] = np.arange(node_slots)
        off = np.zeros(npc + 1, dtype=np.int64)
        np.cumsum(deg, out=off[1:])
        j_e = np.arange(len(e_src)) - off[e_dstl]
        r_e = rank_of[e_dstl]
        col_e = colOf[r_e // P, j_e]
        idx_arrs[i, r_e % P, col_e] = e_src
        rr = np.arange(node_slots)
        real = order < npc
        indg_arrs[i, rr[real] % P, rr[real] // P, 0] = in_dg[i * npc + order[real], 0]
        orders.append(order)

    return dict(
        table=table,
        idx=idx_arrs,
        indg=indg_arrs,
        orders=orders,
        segs=segs,
        C_pad=C_pad,
        G=G,
        npc=npc,
        n=n,
        slots_real=int(sum(len(pc[0]) for pc in per_core)),
        slots_total=int(N_CORES * P * C_pad),
    )


# ---------------------------------------------------------------- device


def _build_nc(n_table_rows, C_pad, G, segs):
    import concourse.bass as bass
    import concourse.tile as tile
    from concourse import bacc, mybir

    nc = bacc.Bacc("TRN2", target_bir_lowering=False, debug=False)
    table_d = nc.dram_tensor(
        "table", [n_table_rows, TW], mybir.dt.float32, kind="ExternalInput"
    ).ap()
    idx_d = nc.dram_tensor("idx", [P, C_pad], mybir.dt.int32, kind="ExternalInput").ap()
    indg_d = nc.dram_tensor(
        "indg", [P, G, 1], mybir.dt.float32, kind="ExternalInput"
    ).ap()
    out_dram = nc.dram_tensor(
        "out", [P, G, D], mybir.dt.float32, kind="ExternalOutput"
    ).ap()

    n_chunks = (C_pad + CHUNK_COLS - 1) // CHUNK_COLS
    segs_by_chunk = {}
    for (ch, cs, w, gs, j, last) in segs:
        segs_by_chunk.setdefault(ch, []).append((cs, w, gs, j, last))

    with tile.TileContext(nc) as tc:
        with (
            tc.tile_pool(name="persist", bufs=1) as persist,
            tc.tile_pool(name="idxp", bufs=4) as idx_pool,
            tc.tile_pool(name="msgs", bufs=4) as msgs_pool,
            tc.tile_pool(name="packed", bufs=3) as packed_pool,
        ):
            indg_t = persist.tile([P, G, 1], mybir.dt.float32)
            acc = persist.tile([P, G, D], mybir.dt.float32)
            nc.sync.dma_start(out=indg_t[:], in_=indg_d[:])
            for ch in range(n_chunks):
                cols = min(CHUNK_COLS, C_pad - ch * CHUNK_COLS)
                # just-in-time index load: chunk 0's gathers start after a
                # 64KB transfer instead of the whole index array
                idx_t = idx_pool.tile([P, cols], mybir.dt.int32, tag="ix")
                nc.sync.dma_start(
                    out=idx_t[:],
                    in_=idx_d[:, ch * CHUNK_COLS : ch * CHUNK_COLS + cols],
                )
                m = msgs_pool.tile([P, cols, TW], mybir.dt.float32, tag="m")
                for c in range(cols):
                    # HW indirect DMA semantics: one descriptor per partition,
                    # row = table[idx[p]] -> dest partition line (2D AP only).
                    nc.gpsimd.indirect_dma_start(
                        out=m[:, c, :],
                        out_offset=None,
                        in_=table_d[:],
                        in_offset=bass.IndirectOffsetOnAxis(
                            ap=idx_t[:, c : c + 1], axis=0
                        ),
                    )
                pk = packed_pool.tile([P, cols, D], mybir.dt.float32, tag="pk")
                nc.vector.tensor_tensor(
                    out=pk[:],
                    in0=m[:, :, 0:D],
                    in1=m[:, :, D : D + 1].to_broadcast([P, cols, D]),
                    op=mybir.AluOpType.mult,
                )
                for (cs, w, gs, j, last) in segs_by_chunk.get(ch, []):
                    if j == 0:
                        # first tile of the group: init the acc region
                        nc.vector.tensor_copy(
                            out=acc[:, gs : gs + w, :], in_=pk[:, cs : cs + w, :]
                        )
                    else:
                        nc.vector.tensor_tensor(
                            out=acc[:, gs : gs + w, :],
                            in0=pk[:, cs : cs + w, :],
                            in1=acc[:, gs : gs + w, :],
                            op=mybir.AluOpType.add,
                        )
                    if last:
                        # final accumulation for groups [gs, gs+w): scale by
                        # in_dg and ship out now, overlapping later chunks
                        nc.vector.tensor_tensor(
                            out=acc[:, gs : gs + w, :],
                            in0=acc[:, gs : gs + w, :],
                            in1=indg_t[:, gs : gs + w, :].to_broadcast([P, w, D]),
                            op=mybir.AluOpType.mult,
                        )
                        nc.sync.dma_start(
                            out=out_dram[:, gs : gs + w, :],
                            in_=acc[:, gs : gs + w, :],
                        )
    nc.compile()
    return nc


# ---------------------------------------------------------------- entry


last_run_info = {}


def kernel(node_f, out_d, in_dg, src, dst, *, _trace=False):
    node_f = np.asarray(node_f, dtype=np.float32)
    out_d = np.asarray(out_d, dtype=np.float32)
    in_dg = np.asarray(in_dg, dtype=np.float32)
    src = np.asarray(src)
    dst = np.asarray(dst)

    pp = _preprocess(node_f, out_d, in_dg, src, dst)

    key = (pp["n"], pp["C_pad"], pp["G"], len(pp["segs"]))
    if key not in _cache:
        _cache.clear()
        _cache[key] = _build_nc(pp["n"] + 1, pp["C_pad"], pp["G"], pp["segs"])
    nc = _cache[key]

    from concourse.bass_utils import run_bass_kernel_spmd

    in_maps = [
        {"table": pp["table"], "idx": pp["idx"][i], "indg": pp["indg"][i]}
        for i in range(N_CORES)
    ]
    # Sacrificial device touch: after an earlier crashed session the first
    # device interaction can report NRT_EXEC_UNIT_UNRECOVERABLE once and
    # then recover; absorb that here instead of failing the real run.
    try:
        import jax
        import jax.numpy as jnp

        jnp.zeros((2,)).block_until_ready()
    except Exception:
        pass

    trace_kwargs = (
        dict(trace=True, trace_cores=list(range(N_CORES))) if _trace else {}
    )
    res = None
    last_exc = None
    for attempt in range(3):
        try:
            res = run_bass_kernel_spmd(
                nc, in_maps, core_ids=list(range(N_CORES)), **trace_kwargs
            )
            break
        except ModuleNotFoundError:
            # NTFF profiling hook unavailable in this environment
            trace_kwargs = {}
        except Exception as e:  # noqa: BLE001
            last_exc = e
            import time as _time

            _time.sleep(2.0)
    if res is None:
        res = run_bass_kernel_spmd(nc, in_maps, core_ids=list(range(N_CORES)))
    last_run_info["exec_time_ns"] = res.exec_time_ns
    last_run_info["mean_exec_time_ns"] = res.mean_exec_time_ns
    last_run_info["trace"] = res.instructions_and_trace
    last_run_info["pp_stats"] = {
        k: pp[k] for k in ("C_pad", "G", "slots_real", "slots_total")
    }

    n, npc, G = pp["n"], pp["npc"], pp["G"]
    out = np.empty((n, D), dtype=np.float32)
    rr = np.arange(G * P)
    for i in range(N_CORES):
        o = res.results[i]["out"]  # [P, G, D]
        order = pp["orders"][i]
        real = order < npc
        out[i * npc + order[real]] = o[rr[real] % P, rr[real] // P]
    return out


# revision 24
# speedup vs baseline: 1.0490x; 1.0015x over previous
"""GCN layer (gather + segment_sum + scale) on 8 Trainium2 NeuronCores.

Strategy (1D destination-node parallel):
  - Host (integer/index work only): shard edges by dst block of 12500 nodes
    (core i owns dst nodes [12500*i, 12500*(i+1))). Per core, sort owned
    nodes by in-degree, pack them into 98 groups of 128 nodes. Group g gets
    k_g "slot tiles": slot (p, c) holds the j-th in-edge of node rank
    g*128+p, padded with a pointer to an all-zero table row when j >= deg.
    Groups with equal k are batched so device-side adds are wide.
  - Device (all FP math): table rows are [node_f || out_d] (33 f32).
    Chunked indirect-DMA gather (SWDGE) of 128-column slot blocks ->
    DVE scale-by-out_d (pack to 32 wide) -> DVE wide accumulate into an
    SBUF accumulator [128, 98, 32] -> multiply by in_dg -> DMA out.
  - Host: inverse-permute rows back to original node order.
"""

import sys

import numpy as np

for _p in ("/opt/trn_rl_repo", "/root/.axon_site/_ro/trn_rl_repo"):
    if _p not in sys.path:
        sys.path.append(_p)

P = 128
D = 32
TW = D + 1  # table row width: 32 features + out_d
CHUNK_COLS = 128
N_CORES = 8

_cache = {}


# ---------------------------------------------------------------- host prep


def _segments(k_g, chunk_cols):
    """List of (chunk, cs_local, w, gs, j): add (or copy when j==0)
    pk[chunk][:, cs:cs+w, :] into acc[:, gs:gs+w, :]."""
    G = len(k_g)
    runs = []
    g0 = 0
    while g0 < G:
        g1 = g0
        while g1 < G and k_g[g1] == k_g[g0]:
            g1 += 1
        runs.append((g0, g1, int(k_g[g0])))
        g0 = g1
    segs = []
    c = 0
    colbase = {}
    # emit largest-k batches first so the schedule tail is a small batch's
    # finalize, not the big one's
    for (g0, g1, k) in reversed(runs):
        b = g1 - g0
        for j in range(k):
            colbase[(g0, j)] = c
            s = c
            while s < c + b:
                e = min(c + b, (s // chunk_cols + 1) * chunk_cols)
                # last=True marks the final accumulation into this group range:
                # finalize (in_dg multiply + output DMA) can fire right after.
                segs.append(
                    (s // chunk_cols, s % chunk_cols, e - s, g0 + (s - c), j, j == k - 1)
                )
                s = e
            c += b
    return segs, colbase, c


def _preprocess(node_f, out_d, in_dg, src, dst):
    n = node_f.shape[0]
    npc = n // N_CORES  # nodes per core
    G = (npc + P - 1) // P
    node_slots = G * P

    table = np.zeros((n + 1, TW), dtype=np.float32)
    table[:n, :D] = node_f
    table[:n, D] = out_d[:, 0]

    core_of = dst // npc
    per_core = []
    deg_all = []
    for i in range(N_CORES):
        m = core_of == i
        e_src = src[m].astype(np.int64)
        e_dstl = (dst[m] - i * npc).astype(np.int64)
        # group edges by local dst
        perm = np.argsort(e_dstl, kind="stable")
        e_src = e_src[perm]
        e_dstl = e_dstl[perm]
        deg = np.bincount(e_dstl, minlength=npc)
        deg_ext = np.zeros(node_slots, dtype=np.int64)
        deg_ext[:npc] = deg
        order = np.argsort(deg_ext, kind="stable")  # rank -> node slot
        per_core.append((e_src, e_dstl, deg, deg_ext, order))
        deg_all.append(deg_ext[order])  # sorted degrees

    # global slot count per group = max over cores of group-max degree
    k_g = np.zeros(G, dtype=np.int64)
    for i in range(N_CORES):
        sd = deg_all[i]
        k_g = np.maximum(k_g, sd.reshape(G, P).max(axis=1))
    # >=1 so every acc region gets its j==0 copy (first-touch init)
    k_g = np.maximum(k_g, 1)

    segs, colbase, C = _segments(k_g, CHUNK_COLS)
    C_pad = C  # no chunk rounding: the last chunk is emitted partial-width

    # colOf lookup table for vectorized idx fill
    kmax = int(k_g.max())
    colOf = np.full((G, kmax), C_pad - 1, dtype=np.int64)  # default -> harmless
    g = 0
    while g < G:
        g1 = g
        while g1 < G and k_g[g1] == k_g[g]:
            g1 += 1
        for j in range(int(k_g[g])):
            colOf[g:g1, j] = colbase[(g, j)] + np.arange(g1 - g)
        g = g1

    idx_arrs = np.full((N_CORES, P, C_pad), n, dtype=np.int32)  # n = zero row
    indg_arrs = np.zeros((N_CORES, P, G, 1), dtype=np.float32)
    orders = []
    for i in range(N_CORES):
        e_src, e_dstl, deg, deg_ext, order = per_core[i]
        rank_of = np.empty(node_slots, dtype=np.int64)
        rank_of[order] = np.arange(node_slots)
        off = np.zeros(npc + 1, dtype=np.int64)
        np.cumsum(deg, out=off[1:])
        j_e = np.arange(len(e_src)) - off[e_dstl]
        r_e = rank_of[e_dstl]
        col_e = colOf[r_e // P, j_e]
        idx_arrs[i, r_e % P, col_e] = e_src
        rr = np.arange(node_slots)
        real = order < npc
        indg_arrs[i, rr[real] % P, rr[real] // P, 0] = in_dg[i * npc + order[real], 0]
        orders.append(order)

    return dict(
        table=table,
        idx=idx_arrs,
        indg=indg_arrs,
        orders=orders,
        segs=segs,
        C_pad=C_pad,
        G=G,
        npc=npc,
        n=n,
        slots_real=int(sum(len(pc[0]) for pc in per_core)),
        slots_total=int(N_CORES * P * C_pad),
    )


# ---------------------------------------------------------------- device


def _build_nc(n_table_rows, C_pad, G, segs):
    import concourse.bass as bass
    import concourse.tile as tile
    from concourse import bacc, mybir

    nc = bacc.Bacc("TRN2", target_bir_lowering=False, debug=False)
    table_d = nc.dram_tensor(
        "table", [n_table_rows, TW], mybir.dt.float32, kind="ExternalInput"
    ).ap()
    idx_d = nc.dram_tensor("idx", [P, C_pad], mybir.dt.int32, kind="ExternalInput").ap()
    indg_d = nc.dram_tensor(
        "indg", [P, G, 1], mybir.dt.float32, kind="ExternalInput"
    ).ap()
    out_dram = nc.dram_tensor(
        "out", [P, G, D], mybir.dt.float32, kind="ExternalOutput"
    ).ap()

    n_chunks = (C_pad + CHUNK_COLS - 1) // CHUNK_COLS
    segs_by_chunk = {}
    for (ch, cs, w, gs, j, last) in segs:
        segs_by_chunk.setdefault(ch, []).append((cs, w, gs, j, last))

    with tile.TileContext(nc) as tc:
        with (
            tc.tile_pool(name="persist", bufs=1) as persist,
            tc.tile_pool(name="idxp", bufs=4) as idx_pool,
            tc.tile_pool(name="msgs", bufs=4) as msgs_pool,
            tc.tile_pool(name="packed", bufs=3) as packed_pool,
        ):
            indg_t = persist.tile([P, G, 1], mybir.dt.float32)
            acc = persist.tile([P, G, D], mybir.dt.float32)
            nc.sync.dma_start(out=indg_t[:], in_=indg_d[:])
            for ch in range(n_chunks):
                cols = min(CHUNK_COLS, C_pad - ch * CHUNK_COLS)
                # just-in-time index load: chunk 0's gathers start after a
                # 64KB transfer instead of the whole index array
                idx_t = idx_pool.tile([P, cols], mybir.dt.int32, tag="ix")
                nc.sync.dma_start(
                    out=idx_t[:],
                    in_=idx_d[:, ch * CHUNK_COLS : ch * CHUNK_COLS + cols],
                )
                m = msgs_pool.tile([P, cols, TW], mybir.dt.float32, tag="m")
                for c in range(cols):
                    # HW indirect DMA semantics: one descriptor per partition,
                    # row = table[idx[p]] -> dest partition line (2D AP only).
                    nc.gpsimd.indirect_dma_start(
                        out=m[:, c, :],
                        out_offset=None,
                        in_=table_d[:],
                        in_offset=bass.IndirectOffsetOnAxis(
                            ap=idx_t[:, c : c + 1], axis=0
                        ),
                    )
                pk = packed_pool.tile([P, cols, D], mybir.dt.float32, tag="pk")
                nc.vector.tensor_tensor(
                    out=pk[:],
                    in0=m[:, :, 0:D],
                    in1=m[:, :, D : D + 1].to_broadcast([P, cols, D]),
                    op=mybir.AluOpType.mult,
                )
                for (cs, w, gs, j, last) in segs_by_chunk.get(ch, []):
                    if j == 0:
                        # first tile of the group: init the acc region
                        nc.vector.tensor_copy(
                            out=acc[:, gs : gs + w, :], in_=pk[:, cs : cs + w, :]
                        )
                    else:
                        nc.vector.tensor_tensor(
                            out=acc[:, gs : gs + w, :],
                            in0=pk[:, cs : cs + w, :],
                            in1=acc[:, gs : gs + w, :],
                            op=mybir.AluOpType.add,
                        )
                    if last:
                        # final accumulation for groups [gs, gs+w): scale by
                        # in_dg and ship out now, overlapping later chunks
                        nc.vector.tensor_tensor(
                            out=acc[:, gs : gs + w, :],
                            in0=acc[:, gs : gs + w, :],
                            in1=indg_t[:, gs : gs + w, :].to_broadcast([P, w, D]),
                            op=mybir.AluOpType.mult,
                        )
                        nc.sync.dma_start(
                            out=out_dram[:, gs : gs + w, :],
                            in_=acc[:, gs : gs + w, :],
                        )
    nc.compile()
    return nc


# ---------------------------------------------------------------- entry


last_run_info = {}


def kernel(node_f, out_d, in_dg, src, dst, *, _trace=False):
    node_f = np.asarray(node_f, dtype=np.float32)
    out_d = np.asarray(out_d, dtype=np.float32)
    in_dg = np.asarray(in_dg, dtype=np.float32)
    src = np.asarray(src)
    dst = np.asarray(dst)

    pp = _preprocess(node_f, out_d, in_dg, src, dst)

    key = (pp["n"], pp["C_pad"], pp["G"], len(pp["segs"]))
    if key not in _cache:
        _cache.clear()
        _cache[key] = _build_nc(pp["n"] + 1, pp["C_pad"], pp["G"], pp["segs"])
    nc = _cache[key]

    from concourse.bass_utils import run_bass_kernel_spmd

    in_maps = [
        {"table": pp["table"], "idx": pp["idx"][i], "indg": pp["indg"][i]}
        for i in range(N_CORES)
    ]
    # Sacrificial device touch: after an earlier crashed session the first
    # device interaction can report NRT_EXEC_UNIT_UNRECOVERABLE once and
    # then recover; absorb that here instead of failing the real run.
    try:
        import jax
        import jax.numpy as jnp

        jnp.zeros((2,)).block_until_ready()
    except Exception:
        pass

    trace_kwargs = (
        dict(trace=True, trace_cores=list(range(N_CORES))) if _trace else {}
    )
    res = None
    last_exc = None
    for attempt in range(3):
        try:
            res = run_bass_kernel_spmd(
                nc, in_maps, core_ids=list(range(N_CORES)), **trace_kwargs
            )
            break
        except ModuleNotFoundError:
            # NTFF profiling hook unavailable in this environment
            trace_kwargs = {}
        except Exception as e:  # noqa: BLE001
            last_exc = e
            import time as _time

            _time.sleep(2.0)
    if res is None:
        res = run_bass_kernel_spmd(nc, in_maps, core_ids=list(range(N_CORES)))
    last_run_info["exec_time_ns"] = res.exec_time_ns
    last_run_info["mean_exec_time_ns"] = res.mean_exec_time_ns
    last_run_info["trace"] = res.instructions_and_trace
    last_run_info["pp_stats"] = {
        k: pp[k] for k in ("C_pad", "G", "slots_real", "slots_total")
    }

    n, npc, G = pp["n"], pp["npc"], pp["G"]
    out = np.empty((n, D), dtype=np.float32)
    rr = np.arange(G * P)
    for i in range(N_CORES):
        o = res.results[i]["out"]  # [P, G, D]
        order = pp["orders"][i]
        real = order < npc
        out[i * npc + order[real]] = o[rr[real] % P, rr[real] // P]
    return out
